# revision 3
# baseline (speedup 1.0000x reference)
"""Multi-head masked self-attention on 8 trn2 NeuronCores.

Problem: B=2, T=2048, H=1024, nH=16 heads (head_dim=64), causal softmax
attention with QKV projections; scores scaled by 1/sqrt(H).

Sharding: heads across cores (2 heads per core), both batches on every core
(B*nH = 32 (b,h) pairs -> 4 per core). QKV weights column-sharded by head:
core m gets W[128m:128m+128, :] of each projection matrix.

Per-core device program (all matmuls in float32r: fp32 range, 11-bit
mantissa, full PE rate at free-dim >= 256; inputs pre-rounded on host so the
device matmuls are exact on the rounded values):

  xT[b] (host-pretransposed [1024, 2048]) -> SBUF [128, 8cb, 2048]
  Q^T/K^T per head [64, 2048] = W_h @ xT  (PE, accumulate over 8 c-blocks,
      evicted from PSUM by DVE tensor_scalar_add with per-partition bias)
  V^T per head [64, 2048] likewise; PE-transposed 128 cols at a time into
      V' [128k, 16kb, 65] with a ones-column appended (col 64).
  Scores (transposed): S^T[k, q] = K^T.T(slice) @ Q^T -> PSUM [128, 512]
      causal: for each q-tile of 512, k-blocks 0..4qt+3; diagonal blocks
      computed on the live q-slice only, additive -1e6 triangle mask on the
      leading 128-col block (DVE on PSUM), then ACT evicts exp(S/32) -> f32r.
      No row-max subtraction: |S| < 1 for this input distribution.
  O'^T [65, 512] = sum_k V'[kb].T @ P^T[kb]  (PE accumulation in one PSUM
      bank; row 64 accumulates the softmax denominator Z).
  O'^T -> (PE transpose per 128 q) -> [128, 65]; DVE reciprocal of Z col;
      ACT Copy with per-partition scale -> out tile [128, 64] -> DMA out.
"""
import sys

sys.path.insert(0, "/opt/trn_rl_repo")

import numpy as np

B = 2
T = 2048
H = 1024
NHEADS = 16
HD = 64
NCORES = 8
HEADS_PER_CORE = NHEADS // NCORES  # 2
P = 128
CB = H // P            # 8 contraction blocks for projections
QTILE = 512
NQT = T // QTILE       # 4 q-tiles
NKB = T // P           # 16 k-blocks
MASK_VAL = -1.0e6      # additive causal mask (exp(MASK_VAL/32) == 0 on HW)
SCALE = 1.0 / np.sqrt(np.float32(H))  # 1/32


def _round_fp32r(x: np.ndarray) -> np.ndarray:
    """Round fp32 to the fp32r grid (11-bit mantissa), round-to-nearest-even.

    Matches the PE's fp32r operand rounding, so device matmuls on these
    values are exact (products of 12-bit-significand values fit fp32).
    """
    b = np.ascontiguousarray(x, np.float32).view(np.uint32)
    drop = 12
    bias = np.uint32((1 << (drop - 1)) - 1) + ((b >> drop) & 1)
    r = (b + bias) & np.uint32(~((1 << drop) - 1) & 0xFFFFFFFF)
    return r.view(np.float32)


def _build_program():
    import concourse.tile as tile
    from concourse import bacc, mybir
    from concourse.masks import make_identity
    from concourse.bass import ts

    F32 = mybir.dt.float32
    F32R = mybir.dt.float32r
    ActF = mybir.ActivationFunctionType

    nc = bacc.Bacc("TRN2", target_bir_lowering=False, debug=False)

    xt_d = nc.dram_tensor("xt", [B, H, T], F32R, kind="ExternalInput")
    w_d = {
        n: nc.dram_tensor(f"w{n}t", [H, P], F32R, kind="ExternalInput")
        for n in "qkv"
    }
    b_d = {
        n: nc.dram_tensor(f"b{n}", [P], F32, kind="ExternalInput")
        for n in "qkv"
    }
    out_d = nc.dram_tensor("out", [B, T, P], F32, kind="ExternalOutput")

    with tile.TileContext(nc) as tc:
        with (
            tc.tile_pool(name="const", bufs=1) as const,
            tc.tile_pool(name="xt", bufs=1) as xt_pool,
            tc.tile_pool(name="qkv", bufs=1) as qkv_pool,
            tc.tile_pool(name="vp", bufs=2) as vp_pool,
            tc.tile_pool(name="pt", bufs=5) as pt_pool,
            tc.tile_pool(name="osb", bufs=3) as osb_pool,
            tc.tile_pool(name="fin", bufs=4) as fin_pool,
            tc.tile_pool(name="psmm", bufs=4, space="PSUM") as psmm,
            tc.tile_pool(name="pso", bufs=2, space="PSUM") as pso,
            tc.tile_pool(name="pstr", bufs=2, space="PSUM") as pstr,
        ):
            # ---- constants ----
            ident = const.tile([P, P], F32, tag="ident")
            make_identity(nc, ident[:])
            # additive causal triangle for the diagonal 128x128 block of a
            # k-block: keep (0) where k_local <= q_local else MASK_VAL
            trimask = const.tile([P, P], F32, tag="trimask")
            nc.gpsimd.memset(trimask[:], 0.0)
            nc.gpsimd.affine_select(
                out=trimask[:],
                in_=trimask[:],
                compare_op=mybir.AluOpType.is_ge,
                fill=MASK_VAL,
                base=0,
                pattern=[[1, P]],
                channel_multiplier=-1,
            )
            ones16 = const.tile([P, NKB], F32, tag="ones16")
            nc.vector.memset(ones16[:], 1.0)

            w_sb = {}
            bias_sb = {}
            for n in "qkv":
                w_sb[n] = const.tile([P, CB, P], F32R, tag=f"w{n}", name=f"w{n}")
                nc.sync.dma_start(
                    w_sb[n][:],
                    w_d[n][:].rearrange("(cb p) m -> p cb m", p=P),
                )
                bias_sb[n] = const.tile([P, 1], F32, tag=f"b{n}", name=f"b{n}")
                nc.sync.dma_start(bias_sb[n][:], b_d[n][:, None])

            for b in range(B):
                # ---- load x^T for this batch ----
                xt = xt_pool.tile([P, CB, T], F32R, tag="xt")
                for cb in range(CB):
                    nc.sync.dma_start(xt[:, cb, :], xt_d[b, ts(cb, P), :])

                # ---- QKV projections ----
                # per-head Q^T/K^T f32r [64, T]; per-head V^T fp32 [64, T]
                qt_h = [qkv_pool.tile([HD, T], F32R, tag=f"qt{h}", name=f"qt{h}") for h in range(2)]
                kt_h = [qkv_pool.tile([HD, T], F32R, tag=f"kt{h}", name=f"kt{h}") for h in range(2)]
                vt_h = [qkv_pool.tile([HD, T], F32, tag=f"vt{h}", name=f"vt{h}") for h in range(2)]
                for n, dsts in (("q", qt_h), ("k", kt_h), ("v", vt_h)):
                    for tt in range(NQT):
                        psA = psmm.tile([P, QTILE], F32, tag="mm")
                        for cb in range(CB):
                            nc.tensor.matmul(
                                psA[:],
                                w_sb[n][:, cb, :],
                                xt[:, cb, ts(tt, QTILE)],
                                start=(cb == 0),
                                stop=(cb == CB - 1),
                            )
                        for h in range(2):
                            nc.vector.tensor_scalar_add(
                                dsts[h][:, ts(tt, QTILE)],
                                psA[ts(h, HD), :],
                                bias_sb[n][ts(h, HD), :],
                            )

                # ---- V' = [V, 1] with k on partitions, per head ----
                vprime = []
                for h in range(2):
                    vp = vp_pool.tile([P, NKB, HD + 1], F32R, tag="vp")
                    nc.vector.tensor_copy(vp[:, :, HD], ones16[:])
                    for kb in range(NKB):
                        trp = pstr.tile([P, P], F32, tag="tr")
                        nc.tensor.transpose(
                            trp[:, :HD], vt_h[h][:, ts(kb, P)], ident[:HD, :HD]
                        )
                        nc.vector.tensor_copy(vp[:, kb, :HD], trp[:, :HD])
                    vprime.append(vp)

                # ---- attention per head ----
                for h in range(2):
                    for qt in range(NQT):
                        psO = pso.tile([P, QTILE], F32, tag="o")
                        nkb = 4 * qt + 4
                        for kb in range(nkb):
                            i = kb - 4 * qt  # >= 0 on the diagonal region
                            lo = max(i, 0) * P  # live q-slice start in tile
                            width = QTILE - lo
                            psS = psmm.tile([P, QTILE], F32, tag="mm")
                            nc.tensor.matmul(
                                psS[:, lo:QTILE],
                                kt_h[h][:, ts(kb, P)],
                                qt_h[h][:, qt * QTILE + lo : (qt + 1) * QTILE],
                            )
                            if i >= 0:
                                nc.vector.tensor_add(
                                    psS[:, lo : lo + P],
                                    psS[:, lo : lo + P],
                                    trimask[:],
                                )
                            pt = pt_pool.tile([P, QTILE], F32R, tag="pt")
                            nc.scalar.activation(
                                pt[:, lo:QTILE],
                                psS[:, lo:QTILE],
                                ActF.Exp,
                                scale=float(SCALE),
                            )
                            nc.tensor.matmul(
                                psO[: HD + 1, lo:QTILE],
                                vprime[h][:, kb, :],
                                pt[:, lo:QTILE],
                                start=(kb == 0),
                                stop=(kb == nkb - 1),
                            )
                        # ---- evict, transpose, normalize, store ----
                        oT = osb_pool.tile([HD + 1, QTILE], F32, tag="oT")
                        nc.vector.tensor_copy(oT[:], psO[: HD + 1, :])
                        for j in range(4):
                            trp = pstr.tile([P, P], F32, tag="tr")
                            nc.tensor.transpose(
                                trp[:, : HD + 1],
                                oT[:, ts(j, P)],
                                ident[: HD + 1, : HD + 1],
                            )
                            ot = fin_pool.tile([P, HD + 1], F32, tag="ot")
                            nc.vector.tensor_copy(ot[:], trp[:, : HD + 1])
                            rec = fin_pool.tile([P, 1], F32, tag="rec")
                            nc.vector.reciprocal(rec[:], ot[:, HD : HD + 1])
                            fin = fin_pool.tile([P, HD], F32, tag="fin")
                            nc.scalar.activation(
                                fin[:], ot[:, :HD], ActF.Copy, scale=rec[:]
                            )
                            nc.sync.dma_start(
                                out_d[b, qt * QTILE + j * P : qt * QTILE + (j + 1) * P, ts(h, HD)],
                                fin[:],
                            )

    nc.compile()
    return nc


_CACHED = {}


def kernel(x, Wq, bq, Wk, bk, Wv, bv):
    from concourse.bass_utils import run_bass_kernel_spmd

    x = np.ascontiguousarray(np.asarray(x, np.float32))
    # host-side prep: transpose x to [B, H, T], pre-round matmul operands
    xt = _round_fp32r(np.ascontiguousarray(x.transpose(0, 2, 1)))

    if "nc" not in _CACHED:
        _CACHED["nc"] = _build_program()
    nc = _CACHED["nc"]

    in_maps = []
    for m in range(NCORES):
        sl = slice(m * P, (m + 1) * P)  # 128 output channels = 2 heads
        in_maps.append({
            "xt": xt,
            "wqt": _round_fp32r(np.asarray(Wq)[sl, :].T),
            "wkt": _round_fp32r(np.asarray(Wk)[sl, :].T),
            "wvt": _round_fp32r(np.asarray(Wv)[sl, :].T),
            "bq": np.ascontiguousarray(np.asarray(bq, np.float32)[sl]),
            "bk": np.ascontiguousarray(np.asarray(bk, np.float32)[sl]),
            "bv": np.ascontiguousarray(np.asarray(bv, np.float32)[sl]),
        })

    res = run_bass_kernel_spmd(nc, in_maps, core_ids=list(range(NCORES)))
    out = np.concatenate(
        [res.results[m]["out"] for m in range(NCORES)], axis=-1
    )
    return out


# revision 4
# speedup vs baseline: 3.1606x; 3.1606x over previous
"""Multi-head masked self-attention on 8 trn2 NeuronCores.

Problem: B=2, T=2048, H=1024, nH=16 heads (head_dim=64), causal softmax
attention with QKV projections; scores scaled by 1/sqrt(H).

Sharding: heads across cores (2 heads per core), both batches on every core
(B*nH = 32 (b,h) pairs -> 4 per core). QKV weights column-sharded by head:
core m gets W[128m:128m+128, :] of each projection matrix.

Per-core device program (all matmuls in float32r: fp32 range, 11-bit
mantissa, full PE rate at free-dim >= 256; inputs pre-rounded on host so the
device matmuls are exact on the rounded values):

  xT[b] (host-pretransposed [1024, 2048]) -> SBUF [128, 8cb, 2048]
  Q^T/K^T per head [64, 2048] = W_h @ xT  (PE, accumulate over 8 c-blocks,
      evicted from PSUM by DVE tensor_scalar_add with per-partition bias)
  V^T per head [64, 2048] likewise; PE-transposed 128 cols at a time into
      V' [128k, 16kb, 65] with a ones-column appended (col 64).
  Scores (transposed): S^T[k, q] = K^T.T(slice) @ Q^T -> PSUM [128, 512]
      causal: for each q-tile of 512, k-blocks 0..4qt+3; diagonal blocks
      computed on the live q-slice only, additive -1e6 triangle mask on the
      leading 128-col block (DVE on PSUM), then ACT evicts exp(S/32) -> f32r.
      No row-max subtraction: |S| < 1 for this input distribution.
  O'^T [65, 512] = sum_k V'[kb].T @ P^T[kb]  (PE accumulation in one PSUM
      bank; row 64 accumulates the softmax denominator Z).
  O'^T -> (PE transpose per 128 q) -> [128, 65]; DVE reciprocal of Z col;
      ACT Copy with per-partition scale -> out tile [128, 64] -> DMA out.
"""
import sys

sys.path.insert(0, "/opt/trn_rl_repo")

import numpy as np

B = 2
T = 2048
H = 1024
NHEADS = 16
HD = 64
NCORES = 8
HEADS_PER_CORE = NHEADS // NCORES  # 2
P = 128
CB = H // P            # 8 contraction blocks for projections
QTILE = 512
NQT = T // QTILE       # 4 q-tiles
NKB = T // P           # 16 k-blocks
MASK_VAL = -1.0e6      # additive causal mask (exp(MASK_VAL/32) == 0 on HW)
SCALE = 1.0 / np.sqrt(np.float32(H))  # 1/32


def _round_fp32r(x: np.ndarray) -> np.ndarray:
    """Round fp32 to the fp32r grid (11-bit mantissa), round-to-nearest-even.

    Matches the PE's fp32r operand rounding, so device matmuls on these
    values are exact (products of 12-bit-significand values fit fp32).
    """
    b = np.ascontiguousarray(x, np.float32).view(np.uint32)
    drop = 12
    bias = np.uint32((1 << (drop - 1)) - 1) + ((b >> drop) & 1)
    r = (b + bias) & np.uint32(~((1 << drop) - 1) & 0xFFFFFFFF)
    return r.view(np.float32)


def _build_program(reps: int = 1):
    import contextlib
    import concourse.tile as tile
    from concourse import bacc, mybir
    from concourse.masks import make_identity
    from concourse.bass import ts

    F32 = mybir.dt.float32
    F32R = mybir.dt.float32r
    ActF = mybir.ActivationFunctionType

    nc = bacc.Bacc("TRN2", target_bir_lowering=False, debug=False)

    xt_d = nc.dram_tensor("xt", [B, H, T], F32R, kind="ExternalInput")
    w_d = {
        n: nc.dram_tensor(f"w{n}t", [H, P], F32R, kind="ExternalInput")
        for n in "qkv"
    }
    b_d = {
        n: nc.dram_tensor(f"b{n}", [P], F32, kind="ExternalInput")
        for n in "qkv"
    }
    out_d = nc.dram_tensor("out", [B, T, P], F32, kind="ExternalOutput")

    with tile.TileContext(nc) as tc:
        with (
            tc.tile_pool(name="const", bufs=1) as const,
            tc.tile_pool(name="xt", bufs=1) as xt_pool,
            tc.tile_pool(name="qkv", bufs=1) as qkv_pool,
            tc.tile_pool(name="vp", bufs=2) as vp_pool,
            tc.tile_pool(name="pt", bufs=5) as pt_pool,
            tc.tile_pool(name="osb", bufs=3) as osb_pool,
            tc.tile_pool(name="fin", bufs=4) as fin_pool,
            tc.tile_pool(name="psmm", bufs=4, space="PSUM") as psmm,
            tc.tile_pool(name="pso", bufs=2, space="PSUM") as pso,
            tc.tile_pool(name="pstr", bufs=2, space="PSUM") as pstr,
        ):
            # ---- constants ----
            ident = const.tile([P, P], F32, tag="ident")
            make_identity(nc, ident[:])
            # additive causal triangle for the diagonal 128x128 block of a
            # k-block: keep (0) where k_local <= q_local else MASK_VAL
            trimask = const.tile([P, P], F32, tag="trimask")
            nc.gpsimd.memset(trimask[:], 0.0)
            nc.gpsimd.affine_select(
                out=trimask[:],
                in_=trimask[:],
                compare_op=mybir.AluOpType.is_ge,
                fill=MASK_VAL,
                base=0,
                pattern=[[1, P]],
                channel_multiplier=-1,
            )
            ones16 = const.tile([P, NKB], F32, tag="ones16")
            nc.vector.memset(ones16[:], 1.0)

            w_sb = {}
            bias_sb = {}
            for n in "qkv":
                w_sb[n] = const.tile([P, CB, P], F32R, tag=f"w{n}", name=f"w{n}")
                nc.sync.dma_start(
                    w_sb[n][:],
                    w_d[n][:].rearrange("(cb p) m -> p cb m", p=P),
                )
                bias_sb[n] = const.tile([P, 1], F32, tag=f"b{n}", name=f"b{n}")
                nc.sync.dma_start(bias_sb[n][:], b_d[n][:, None])

            rep_ctx = tc.For_i(0, reps, 1) if reps > 1 else contextlib.nullcontext()
            with rep_ctx:
              for b in range(B):
                # ---- load x^T for this batch ----
                xt = xt_pool.tile([P, CB, T], F32R, tag="xt")
                for cb in range(CB):
                    nc.sync.dma_start(xt[:, cb, :], xt_d[b, ts(cb, P), :])

                # ---- QKV projections ----
                # per-head Q^T/K^T f32r [64, T]; per-head V^T fp32 [64, T]
                qt_h = [qkv_pool.tile([HD, T], F32R, tag=f"qt{h}", name=f"qt{h}") for h in range(2)]
                kt_h = [qkv_pool.tile([HD, T], F32R, tag=f"kt{h}", name=f"kt{h}") for h in range(2)]
                vt_h = [qkv_pool.tile([HD, T], F32, tag=f"vt{h}", name=f"vt{h}") for h in range(2)]
                for n, dsts in (("q", qt_h), ("k", kt_h), ("v", vt_h)):
                    for tt in range(NQT):
                        psA = psmm.tile([P, QTILE], F32, tag="mm")
                        for cb in range(CB):
                            nc.tensor.matmul(
                                psA[:],
                                w_sb[n][:, cb, :],
                                xt[:, cb, ts(tt, QTILE)],
                                start=(cb == 0),
                                stop=(cb == CB - 1),
                            )
                        for h in range(2):
                            nc.vector.tensor_scalar_add(
                                dsts[h][:, ts(tt, QTILE)],
                                psA[ts(h, HD), :],
                                bias_sb[n][ts(h, HD), :],
                            )

                # ---- V' = [V, 1] with k on partitions, per head ----
                vprime = []
                for h in range(2):
                    vp = vp_pool.tile([P, NKB, HD + 1], F32R, tag="vp")
                    nc.vector.tensor_copy(vp[:, :, HD], ones16[:])
                    for kb in range(NKB):
                        trp = pstr.tile([P, P], F32, tag="tr")
                        nc.tensor.transpose(
                            trp[:, :HD], vt_h[h][:, ts(kb, P)], ident[:HD, :HD]
                        )
                        nc.vector.tensor_copy(vp[:, kb, :HD], trp[:, :HD])
                    vprime.append(vp)

                # ---- attention per head ----
                for h in range(2):
                    for qt in range(NQT):
                        psO = pso.tile([P, QTILE], F32, tag="o")
                        nkb = 4 * qt + 4
                        for kb in range(nkb):
                            i = kb - 4 * qt  # >= 0 on the diagonal region
                            lo = max(i, 0) * P  # live q-slice start in tile
                            width = QTILE - lo
                            psS = psmm.tile([P, QTILE], F32, tag="mm")
                            nc.tensor.matmul(
                                psS[:, lo:QTILE],
                                kt_h[h][:, ts(kb, P)],
                                qt_h[h][:, qt * QTILE + lo : (qt + 1) * QTILE],
                            )
                            if i >= 0:
                                nc.vector.tensor_add(
                                    psS[:, lo : lo + P],
                                    psS[:, lo : lo + P],
                                    trimask[:],
                                )
                            pt = pt_pool.tile([P, QTILE], F32R, tag="pt")
                            nc.scalar.activation(
                                pt[:, lo:QTILE],
                                psS[:, lo:QTILE],
                                ActF.Exp,
                                scale=float(SCALE),
                            )
                            nc.tensor.matmul(
                                psO[: HD + 1, lo:QTILE],
                                vprime[h][:, kb, :],
                                pt[:, lo:QTILE],
                                start=(kb == 0),
                                stop=(kb == nkb - 1),
                            )
                        # ---- evict, transpose, normalize, store ----
                        oT = osb_pool.tile([HD + 1, QTILE], F32, tag="oT")
                        nc.vector.tensor_copy(oT[:], psO[: HD + 1, :])
                        for j in range(4):
                            trp = pstr.tile([P, P], F32, tag="tr")
                            nc.tensor.transpose(
                                trp[:, : HD + 1],
                                oT[:, ts(j, P)],
                                ident[: HD + 1, : HD + 1],
                            )
                            ot = fin_pool.tile([P, HD + 1], F32, tag="ot")
                            nc.vector.tensor_copy(ot[:], trp[:, : HD + 1])
                            rec = fin_pool.tile([P, 1], F32, tag="rec")
                            nc.vector.reciprocal(rec[:], ot[:, HD : HD + 1])
                            fin = fin_pool.tile([P, HD], F32, tag="fin")
                            nc.scalar.activation(
                                fin[:], ot[:, :HD], ActF.Copy, scale=rec[:]
                            )
                            nc.sync.dma_start(
                                out_d[b, qt * QTILE + j * P : qt * QTILE + (j + 1) * P, ts(h, HD)],
                                fin[:],
                            )

    nc.compile()
    return nc


_CACHED = {}


def kernel(x, Wq, bq, Wk, bk, Wv, bv):
    from concourse.bass_utils import run_bass_kernel_spmd

    x = np.ascontiguousarray(np.asarray(x, np.float32))
    # host-side prep: transpose x to [B, H, T], pre-round matmul operands
    xt = _round_fp32r(np.ascontiguousarray(x.transpose(0, 2, 1)))

    if "nc" not in _CACHED:
        _CACHED["nc"] = _build_program()
    nc = _CACHED["nc"]

    in_maps = []
    for m in range(NCORES):
        sl = slice(m * P, (m + 1) * P)  # 128 output channels = 2 heads
        in_maps.append({
            "xt": xt,
            "wqt": _round_fp32r(np.asarray(Wq)[sl, :].T),
            "wkt": _round_fp32r(np.asarray(Wk)[sl, :].T),
            "wvt": _round_fp32r(np.asarray(Wv)[sl, :].T),
            "bq": np.ascontiguousarray(np.asarray(bq, np.float32)[sl]),
            "bk": np.ascontiguousarray(np.asarray(bk, np.float32)[sl]),
            "bv": np.ascontiguousarray(np.asarray(bv, np.float32)[sl]),
        })

    res = run_bass_kernel_spmd(nc, in_maps, core_ids=list(range(NCORES)))
    out = np.concatenate(
        [res.results[m]["out"] for m in range(NCORES)], axis=-1
    )
    return out


# revision 7
# speedup vs baseline: 4.1997x; 1.3287x over previous
"""Multi-head masked self-attention on 8 trn2 NeuronCores.

Problem: B=2, T=2048, H=1024, nH=16 heads (head_dim=64), causal softmax
attention with QKV projections; scores scaled by 1/sqrt(H).

Sharding: heads across cores (2 heads per core), both batches on every core
(B*nH = 32 (b,h) pairs -> 4 per core). QKV weights column-sharded by head:
core m gets W[128m:128m+128, :] of each projection matrix.

Per-core device program (all matmuls in float32r: fp32 range, 11-bit
mantissa, full PE rate at free-dim >= 256; inputs pre-rounded on host so the
device matmuls are exact on the rounded values):

  xT[b] (host-pretransposed [1024, 2048]) -> SBUF [128, 8cb, 2048]
  Q^T/K^T per head [64, 2048] = W_h @ xT  (PE, accumulate over 8 c-blocks,
      evicted from PSUM by DVE tensor_scalar_add with per-partition bias)
  V^T per head [64, 2048] likewise; PE-transposed 128 cols at a time into
      V' [128k, 16kb, 65] with a ones-column appended (col 64).
  Scores (transposed): S^T[k, q] = K^T.T(slice) @ Q^T -> PSUM [128, 512]
      causal: for each q-tile of 512, k-blocks 0..4qt+3; diagonal blocks
      computed on the live q-slice only, additive -1e6 triangle mask on the
      leading 128-col block (DVE on PSUM), then ACT evicts exp(S/32) -> f32r.
      No row-max subtraction: |S| < 1 for this input distribution.
  O'^T [65, 512] = sum_k V'[kb].T @ P^T[kb]  (PE accumulation in one PSUM
      bank; row 64 accumulates the softmax denominator Z).
  O'^T -> (PE transpose per 128 q) -> [128, 65]; DVE reciprocal of Z col;
      ACT Copy with per-partition scale -> out tile [128, 64] -> DMA out.
"""
import sys

sys.path.insert(0, "/opt/trn_rl_repo")

import numpy as np

B = 2
T = 2048
H = 1024
NHEADS = 16
HD = 64
NCORES = 8
HEADS_PER_CORE = NHEADS // NCORES  # 2
P = 128
CB = H // P            # 8 contraction blocks for projections
QTILE = 512
NQT = T // QTILE       # 4 q-tiles
NKB = T // P           # 16 k-blocks
MASK_VAL = -1.0e6      # additive causal mask (exp(MASK_VAL/32) == 0 on HW)
SCALE = 1.0 / np.sqrt(np.float32(H))  # 1/32


def _round_fp32r(x: np.ndarray) -> np.ndarray:
    """Round fp32 to the fp32r grid (11-bit mantissa), round-to-nearest-even.

    Matches the PE's fp32r operand rounding, so device matmuls on these
    values are exact (products of 12-bit-significand values fit fp32).
    """
    b = np.ascontiguousarray(x, np.float32).view(np.uint32)
    drop = 12
    bias = np.uint32((1 << (drop - 1)) - 1) + ((b >> drop) & 1)
    r = (b + bias) & np.uint32(~((1 << drop) - 1) & 0xFFFFFFFF)
    return r.view(np.float32)


def _build_program(reps: int = 1):
    import contextlib
    import concourse.tile as tile
    from concourse import bacc, mybir
    from concourse.masks import make_identity
    from concourse.bass import ts

    F32 = mybir.dt.float32
    F32R = mybir.dt.float32r
    ActF = mybir.ActivationFunctionType

    nc = bacc.Bacc("TRN2", target_bir_lowering=False, debug=False)

    xt_d = nc.dram_tensor("xt", [B, H, T], F32R, kind="ExternalInput")
    w_d = {
        n: nc.dram_tensor(f"w{n}t", [H, P], F32R, kind="ExternalInput")
        for n in "qkv"
    }
    b_d = {
        n: nc.dram_tensor(f"b{n}", [P], F32, kind="ExternalInput")
        for n in "qkv"
    }
    out_d = nc.dram_tensor("out", [B, T, P], F32, kind="ExternalOutput")

    with tile.TileContext(nc) as tc:
        with (
            tc.tile_pool(name="const", bufs=1) as const,
            tc.tile_pool(name="xt", bufs=1) as xt_pool,
            tc.tile_pool(name="qkv", bufs=2) as qkv_pool,
            tc.tile_pool(name="vp", bufs=2) as vp_pool,
            tc.tile_pool(name="pt", bufs=5) as pt_pool,
            tc.tile_pool(name="osb", bufs=3) as osb_pool,
            tc.tile_pool(name="fin", bufs=4) as fin_pool,
            tc.tile_pool(name="psmm", bufs=4, space="PSUM") as psmm,
            tc.tile_pool(name="pso", bufs=1, space="PSUM") as pso,
            tc.tile_pool(name="pstr", bufs=2, space="PSUM") as pstr,
        ):
            # ---- constants ----
            ident = const.tile([P, P], F32, tag="ident")
            make_identity(nc, ident[:])
            # additive causal triangle for the diagonal 128x128 block of a
            # k-block: keep (0) where k_local <= q_local else MASK_VAL
            trimask = const.tile([P, P], F32, tag="trimask")
            nc.gpsimd.memset(trimask[:], 0.0)
            nc.gpsimd.affine_select(
                out=trimask[:],
                in_=trimask[:],
                compare_op=mybir.AluOpType.is_ge,
                fill=MASK_VAL,
                base=0,
                pattern=[[1, P]],
                channel_multiplier=-1,
            )
            ones16 = const.tile([P, NKB], F32, tag="ones16")
            nc.vector.memset(ones16[:], 1.0)

            w_sb = {}
            bias_sb = {}
            for n in "qkv":
                w_sb[n] = const.tile([P, CB, P], F32R, tag=f"w{n}", name=f"w{n}")
                nc.sync.dma_start(
                    w_sb[n][:],
                    w_d[n][:].rearrange("(cb p) m -> p cb m", p=P),
                )
                bias_sb[n] = const.tile([P, 1], F32, tag=f"b{n}", name=f"b{n}")
                nc.sync.dma_start(bias_sb[n][:], b_d[n][:, None])

            rep_ctx = (
                tc.For_i(0, reps, 1,
                         hint_engines=(mybir.EngineType.PE,
                                       mybir.EngineType.Activation,
                                       mybir.EngineType.DVE,
                                       mybir.EngineType.SP))
                if reps > 1 else contextlib.nullcontext()
            )
            with rep_ctx:
              for b in range(B):
                # ---- load x^T for this batch ----
                xt = xt_pool.tile([P, CB, T], F32R, tag="xt")
                for cb in range(CB):
                    nc.sync.dma_start(xt[:, cb, :], xt_d[b, ts(cb, P), :])

                # ---- QKV projections (both heads stacked on partitions) ----
                qt_sb = qkv_pool.tile([P, T], F32R, tag="qt", name="qt_sb")
                kt_sb = qkv_pool.tile([P, T], F32R, tag="kt", name="kt_sb")
                vt_h = [qkv_pool.tile([HD, T], F32, tag=f"vt{h}",
                                      name=f"vt{h}") for h in range(2)]
                for n, dsts in (("q", [qt_sb]), ("k", [kt_sb]), ("v", vt_h)):
                    for tt in range(NQT):
                        psA = psmm.tile([P, QTILE], F32, tag="mm")
                        for cb in range(CB):
                            nc.tensor.matmul(
                                psA[:],
                                w_sb[n][:, cb, :],
                                xt[:, cb, ts(tt, QTILE)],
                                start=(cb == 0),
                                stop=(cb == CB - 1),
                            )
                        if len(dsts) == 1:
                            nc.vector.tensor_scalar_add(
                                dsts[0][:, ts(tt, QTILE)], psA[:], bias_sb[n][:]
                            )
                        else:
                            for h in range(2):
                                nc.vector.tensor_scalar_add(
                                    dsts[h][:, ts(tt, QTILE)],
                                    psA[ts(h, HD), :],
                                    bias_sb[n][ts(h, HD), :],
                                )

                # ---- V' = [V, 1] with k on partitions, per head ----
                vprime = []
                for h in range(2):
                    vp = vp_pool.tile([P, NKB, HD + 1], F32R, tag="vp",
                                      name=f"vp{h}")
                    nc.vector.tensor_copy(vp[:, :, HD], ones16[:])
                    for kb in range(NKB):
                        trp = pstr.tile([P, P], F32, tag="tr")
                        nc.tensor.transpose(
                            trp[:, :HD], vt_h[h][:, ts(kb, P)],
                            ident[:HD, :HD],
                        )
                        nc.vector.tensor_copy(vp[:, kb, :HD], trp[:, :HD])
                    vprime.append(vp)

                # ---- attention, heads interleaved (PE row-group overlap:
                # head h's S-matmul streams rows 64h..64h+64) ----
                for qt in range(NQT):
                    psO = [pso.tile([P, QTILE], F32, tag=f"o{h}",
                                    name=f"psO{h}") for h in range(2)]
                    nkb = 4 * qt + 4
                    for kb in range(nkb):
                        i = kb - 4 * qt  # >= 0 on the diagonal region
                        lo = max(i, 0) * P  # live q-slice start in tile
                        for h in range(2):
                            psS = psmm.tile([P, QTILE], F32, tag="mm",
                                            name="psS")
                            nc.tensor.matmul(
                                psS[:, lo:QTILE],
                                kt_sb[ts(h, HD), ts(kb, P)],
                                qt_sb[ts(h, HD),
                                      qt * QTILE + lo : (qt + 1) * QTILE],
                            )
                            if i >= 0:
                                nc.vector.tensor_add(
                                    psS[:, lo : lo + P],
                                    psS[:, lo : lo + P],
                                    trimask[:],
                                )
                            pt = pt_pool.tile([P, QTILE], F32R, tag="pt")
                            nc.scalar.activation(
                                pt[:, lo:QTILE],
                                psS[:, lo:QTILE],
                                ActF.Exp,
                                scale=float(SCALE),
                            )
                            nc.tensor.matmul(
                                psO[h][: HD + 1, lo:QTILE],
                                vprime[h][:, kb, :],
                                pt[:, lo:QTILE],
                                start=(kb == 0),
                                stop=(kb == nkb - 1),
                            )
                    # ---- evict, transpose, normalize (DVE), batched store ----
                    for h in range(2):
                        oT = osb_pool.tile([HD + 1, QTILE], F32, tag="oT")
                        nc.vector.tensor_copy(oT[:], psO[h][: HD + 1, :])
                        fin = fin_pool.tile([P, 4, HD], F32, tag="fin")
                        for j in range(4):
                            trp = pstr.tile([P, P], F32, tag="tr")
                            nc.tensor.transpose(
                                trp[:, : HD + 1],
                                oT[:, ts(j, P)],
                                ident[: HD + 1, : HD + 1],
                            )
                            ot = fin_pool.tile([P, HD + 1], F32, tag="ot")
                            nc.vector.tensor_copy(ot[:], trp[:, : HD + 1])
                            rec = fin_pool.tile([P, 1], F32, tag="rec")
                            nc.vector.reciprocal(rec[:], ot[:, HD : HD + 1])
                            nc.vector.tensor_scalar_mul(
                                fin[:, j, :], ot[:, :HD], rec[:]
                            )
                        nc.sync.dma_start(
                            out_d[b, ts(qt, QTILE), ts(h, HD)].rearrange(
                                "(j p) c -> p j c", p=P
                            ),
                            fin[:],
                        )

    nc.compile()
    return nc


_CACHED = {}


def kernel(x, Wq, bq, Wk, bk, Wv, bv):
    from concourse.bass_utils import run_bass_kernel_spmd

    x = np.ascontiguousarray(np.asarray(x, np.float32))
    # host-side prep: transpose x to [B, H, T], pre-round matmul operands
    xt = _round_fp32r(np.ascontiguousarray(x.transpose(0, 2, 1)))

    if "nc" not in _CACHED:
        _CACHED["nc"] = _build_program()
    nc = _CACHED["nc"]

    in_maps = []
    for m in range(NCORES):
        sl = slice(m * P, (m + 1) * P)  # 128 output channels = 2 heads
        in_maps.append({
            "xt": xt,
            "wqt": _round_fp32r(np.asarray(Wq)[sl, :].T),
            "wkt": _round_fp32r(np.asarray(Wk)[sl, :].T),
            "wvt": _round_fp32r(np.asarray(Wv)[sl, :].T),
            "bq": np.ascontiguousarray(np.asarray(bq, np.float32)[sl]),
            "bk": np.ascontiguousarray(np.asarray(bk, np.float32)[sl]),
            "bv": np.ascontiguousarray(np.asarray(bv, np.float32)[sl]),
        })

    res = run_bass_kernel_spmd(nc, in_maps, core_ids=list(range(NCORES)))
    out = np.concatenate(
        [res.results[m]["out"] for m in range(NCORES)], axis=-1
    )
    return out
